# revision 45
# baseline (speedup 1.0000x reference)
"""Locally-connected 2D layer on 8 Trainium2 NeuronCores.

Problem: x[128,3,64,64] f32, per-position weights W[60,60,32,75], bias b[60,60,32]
  out[b,o,y,x] = sum_k patches[b,y,x,k] * W[y,x,o,k] + b[y,x,o],  k=(c,dy,dx)

Strategy (spatial sharding over output rows, 8 rows/core, memory-regime;
the kernel is HBM-bound at ~235GB/s aggregate, so byte count is king):
  - Groups of 4 consecutive x positions share one full-array matmul: the
    contraction is the UNION of the 4 patch windows, planes (slot, dx', c)
    with dx' in 0..7 -> 120 partitions.  The stationary [120, 128] holds all
    4 positions' weights ((j,o) columns), so each [128, 128] matmul output is
    fully useful: out[(j,o), b] for 4 x-positions at once.  15 matmuls per
    row, 120 per core, N=128 streaming.  Bias is added on the host.
  - The W slab's structural zeros (dx'-j outside 0..4, 37.5% of it) are NOT
    sent over HBM: for column-group j the valid planes form the contiguous
    (per-slot) band dx' in [j, j+5), so rows 2-7's weights load as dense
    [5 x 15]-partition band DMAs into memset-zeroed pair tiles with a j-major
    free layout; LDWEIGHTS reads a strided [120, j:4, o:32] access pattern.
    Rows 0-1 load zero-padded (no memset on the critical path).
  - dy uses a mod-5 ring of row-slots with the per-row rotation folded into
    the host W layout (np.roll), kept in TWO column-generations (even rows
    read gen A, odd gen B): after row k, two slots of gen k%2 are refreshed
    SBUF->SBUF from staged future rows for row k+2.
  - Queue split by measured rates: the gpsimd SWDGE queue (~300GB/s, and the
    one the Tile scheduler's cost model matches) carries all matmul-gating
    inputs in first-use order plus refreshes and late stores; the two HWDGE
    queues (~115GB/s combined) carry the small future-row slabs and early
    row-pair stores (7680B DRAM lines), which gate nothing.
  - Output is bf16 (host upcasts).  PSUM evacuation alternates vector/scalar.
  - 20 full-size dummy matmuls warm the PE HAM clock-gate during the fill.
"""

import numpy as np

B, C, H, WIDTH = 128, 3, 64, 64
KH = KW = 5
RY = RX = 60
O = 32
NCORES = 8
RPC = 8             # output rows computed per core (8*8=64, last 4 dropped)
INR = RPC + KH - 1  # 12 input rows per core
PADH = NCORES * RPC + KH - 1  # 68
NG = 15             # groups of 4 x-positions per row
NPL = C * 8         # 24 planes per ring slot (dx' in 0..7, c)
KP = KH * NPL       # 120 contraction partitions
FU = NG * B         # 1920 free elems per plane (g, b)
CHUNKS = ((0, 4), (4, 4), (8, 4), (12, 3))  # (first group, n groups) per PSUM chunk
NWARM = 20

_cache = {}


def _build():
    import concourse.bass as bass
    import concourse.bacc as bacc
    import concourse.tile as tile
    import concourse.mybir as mybir

    f32 = mybir.dt.float32
    din = mybir.dt.bfloat16
    nc = bacc.Bacc("TRN2", target_bir_lowering=False, debug=False,
                   num_devices=NCORES)
    ui_d = nc.dram_tensor("ui", [KP, FU], din, kind="ExternalInput")
    uib_d = nc.dram_tensor("uib", [KP, FU], din, kind="ExternalInput")
    ufa_d = nc.dram_tensor("ufa", [KP, FU], din, kind="ExternalInput")
    ufb_d = nc.dram_tensor("ufb", [2 * NPL, FU], din, kind="ExternalInput")
    w0_d = nc.dram_tensor("w0", [KP, 4 * NG * O], din, kind="ExternalInput")
    w1_d = nc.dram_tensor("w1", [KP, 4 * NG * O], din, kind="ExternalInput")
    w2_d = nc.dram_tensor("w2", [KP, 6 * 4 * NG * O], din, kind="ExternalInput")
    oc_d = nc.dram_tensor("oc", [4, O, RPC, NG, B], din, kind="ExternalOutput")

    with tile.TileContext(nc) as tc:
        with (
            tc.tile_pool(name="const", bufs=1) as cpool,
            tc.tile_pool(name="os", bufs=4) as opool,
            tc.tile_pool(name="ps", bufs=4, space=bass.MemorySpace.PSUM) as ppool,
            tc.tile_pool(name="pw", bufs=1, space=bass.MemorySpace.PSUM) as wpool,
        ):
            xp = cpool.tile([KP, 2 * FU], din)        # ring, 2 generations
            ufa = cpool.tile([KP, FU], din)           # future rows 5-9
            ufb = cpool.tile([2 * NPL, FU], din)      # future rows 10-11
            wt0 = cpool.tile([KP, 4 * NG * O], din)
            wt1 = cpool.tile([KP, 4 * NG * O], din)
            wtd = [cpool.tile([KP, 2 * 4 * NG * O], din, name=f"wtd{p}")
                   for p in range(3)]
            dm = cpool.tile([128, 640], din)          # warmup operand

            nc.vector.memset(dm[:], 1.0)

            # gpsimd SWDGE queue, in order of first use
            nc.gpsimd.dma_start(xp[:, 0:FU], ui_d[:])         # gen A rows 0-4
            nc.gpsimd.dma_start(wt0[:], w0_d[:])
            nc.gpsimd.dma_start(xp[:, FU:2 * FU], uib_d[:])   # gen B rows 1-5
            nc.gpsimd.dma_start(wt1[:], w1_d[:])
            for pp in range(3):      # W row pairs (2,3) (4,5) (6,7)
                nc.gpsimd.dma_start(
                    wtd[pp][:],
                    w2_d[:, pp * 2 * 4 * NG * O:(pp + 1) * 2 * 4 * NG * O])
            nc.scalar.dma_start(ufa[:], ufa_d[:])
            nc.scalar.dma_start(ufb[:], ufb_d[:])

            # PE warmup: keep the array genuinely busy (full K/M) during the
            # fill so HAM un-throttles the PE clock before the first real
            # matmul; HAM ignores low-activity (tiny-K) matmuls
            pw = wpool.tile([128, 512], f32)
            for _ in range(NWARM):
                nc.tensor.matmul(pw[:, :], dm[:, 0:128], dm[:, 128:640])

            ot = None
            for k in range(RPC):
                gofs = (k % 2) * FU
                wrow = (wt0, wt1, wtd[0], wtd[0], wtd[1], wtd[1],
                        wtd[2], wtd[2])[k]
                wbase = 0 if k < 2 else (k % 2) * NG * 128
                if k % 2 == 0:
                    ot = opool.tile([128, 2 * FU], din)   # one tile per pair
                oofs = (k % 2) * FU
                for ci, (g0, gn) in enumerate(CHUNKS):
                    pt = ppool.tile([128, 4 * B], f32)
                    for gg in range(gn):
                        g = g0 + gg
                        nc.tensor.matmul(
                            pt[:, gg * B:(gg + 1) * B],
                            wrow[:, wbase + g * 128:wbase + (g + 1) * 128],
                            xp[:, gofs + g * B:gofs + (g + 1) * B],
                        )
                    if ci % 2 == 0:
                        nc.vector.tensor_copy(
                            ot[:, oofs + g0 * B:oofs + (g0 + gn) * B],
                            pt[:, :gn * B])
                    else:
                        nc.scalar.copy(
                            ot[:, oofs + g0 * B:oofs + (g0 + gn) * B],
                            pt[:, :gn * B])
                # ring refresh for row k+2 (same generation): slots k%5 and
                # (k+1)%5 take input rows k+5 and k+6
                if k < RPC - 2:
                    for s, r in ((k % KH, k + KH), ((k + 1) % KH, k + KH + 1)):
                        src = (ufa[(r - KH) * NPL:(r - KH + 1) * NPL, :]
                               if r < 2 * KH else
                               ufb[(r - 2 * KH) * NPL:(r - 2 * KH + 1) * NPL, :])
                        nc.gpsimd.dma_start(
                            xp[s * NPL:(s + 1) * NPL, gofs:gofs + FU], src)
                # stores: DRAM line (j,o) holds all 8 rows contiguously, so a
                # row-pair store moves 7680B per line
                oc_pair = oc_d.rearrange("j o k g b -> (j o) (k g b)")
                p0 = (k - 1) * NG * B
                if k in (1, 3):  # pairs (0,1), (2,3) on the HWDGE queues
                    (nc.sync if k == 1 else nc.scalar).dma_start(
                        oc_pair[:, p0:p0 + 2 * FU], ot[:])
                elif k == 5:     # pair (4,5) on gpsimd behind the input slabs
                    nc.gpsimd.dma_start(oc_pair[:, p0:p0 + 2 * FU], ot[:])
                elif k == 6:
                    nc.gpsimd.dma_start(
                        oc_pair[:, k * NG * B:(k + 1) * NG * B], ot[:, 0:FU])
                elif k == 7:     # split the last store to shorten the tail
                    q0 = k * NG * B
                    nc.gpsimd.dma_start(
                        oc_pair[:, q0:q0 + 8 * B], ot[:, FU:FU + 8 * B])
                    nc.gpsimd.dma_start(
                        oc_pair[:, q0 + 8 * B:q0 + FU], ot[:, FU + 8 * B:2 * FU])

    nc.compile()
    return nc


def _get_nc():
    if "nc" not in _cache:
        _cache["nc"] = _build()
    return _cache["nc"]


def _prep_inputs(x, W, b):
    import ml_dtypes
    bf = ml_dtypes.bfloat16
    x = np.asarray(x, np.float32)
    W = np.asarray(W, np.float32)
    xh = np.zeros((PADH, C, WIDTH, B), np.float32)
    xh[:H] = x.transpose(2, 1, 3, 0)  # [row, c, w, batch]
    # union planes: U[row, dx', c, (g,b)] = xh[row, c, 4g+dx', b], dx' in 0..7
    U = np.zeros((PADH, 8, C, NG, B), np.float32)
    for dxp in range(8):
        U[:, dxp] = xh[:, :, dxp::4][:, :, :NG]
    U = U.reshape(PADH, NPL, FU).astype(bf)

    W5 = W.reshape(RY, RX, O, C, KH, KW)
    rm_i, c_i = np.arange(KH)[:, None], np.arange(C)[None, :]
    in_maps = []
    for i in range(NCORES):
        nk = min(RPC, RY - RPC * i)
        W5c = np.zeros((RPC, NG, 4, O, C, KH, KW), np.float32)
        W5c[:nk] = W5[RPC * i:RPC * i + nk].reshape(nk, NG, 4, O, C, KH, KW)
        A = W5c.transpose(6, 5, 4, 0, 1, 2, 3)  # [dx, dy, c, k, g, j, o]
        S2 = np.empty_like(A)  # slot rm holds dy=(rm-k)%5 -> roll dy by k
        for k in range(RPC):
            S2[:, :, :, k] = np.roll(A[:, :, :, k], k, axis=1)

        # padded slabs [KP partitions (s,dx',c), k, (g, j, o)]
        wp = np.zeros((RPC, KP, NG, 4, O), np.float32)
        for j in range(4):
            for dx in range(KH):
                p_pad = rm_i * NPL + (j + dx) * 3 + c_i   # [5, 3]
                for k in range(RPC):
                    wp[k][p_pad, :, j] = S2[dx, :, :, k, :, j, :] \
                        .transpose(0, 1, 2, 3)
        wp = wp.reshape(RPC, KP, NG * 4 * O).astype(bf)

        Uc = U[RPC * i:RPC * i + INR]  # [12, 24, FU]
        in_maps.append({
            "ui": np.ascontiguousarray(Uc[:KH].reshape(KP, FU)),
            "uib": np.ascontiguousarray(Uc[[5, 1, 2, 3, 4]].reshape(KP, FU)),
            "ufa": np.ascontiguousarray(Uc[KH:2 * KH].reshape(KP, FU)),
            "ufb": np.ascontiguousarray(Uc[2 * KH:].reshape(2 * NPL, FU)),
            "w0": np.ascontiguousarray(wp[0]),
            "w1": np.ascontiguousarray(wp[1]),
            "w2": np.ascontiguousarray(
                wp[2:].transpose(1, 0, 2).reshape(KP, 6 * NG * 4 * O)),
        })
    return in_maps


def kernel(x, W, b):
    from concourse.bass_utils import run_bass_kernel_spmd

    nc = _get_nc()
    in_maps = _prep_inputs(x, W, b)
    br = run_bass_kernel_spmd(nc, in_maps, list(range(NCORES)),
                              **_cache.get("run_kwargs", {}))
    _cache["last_run"] = br
    oc = np.stack([np.asarray(br.results[i]["oc"]) for i in range(NCORES)])
    # oc: [i, j, o, k, g, b] -> out[b, o, y=8i+k, x=4g+j]
    oc = oc.astype(np.float32)
    out = oc.transpose(5, 2, 0, 3, 4, 1)  # [b, o, i, k, g, j]
    out = out.reshape(B, O, NCORES * RPC, RX)
    out = out[:, :, :RY, :] + np.asarray(b, np.float32).transpose(2, 0, 1)[None]
    return np.ascontiguousarray(out)


# revision 46
# speedup vs baseline: 1.1372x; 1.1372x over previous
"""Locally-connected 2D layer on 8 Trainium2 NeuronCores.

Problem: x[128,3,64,64] f32, per-position weights W[60,60,32,75], bias b[60,60,32]
  out[b,o,y,x] = sum_k patches[b,y,x,k] * W[y,x,o,k] + b[y,x,o],  k=(c,dy,dx)

Strategy (spatial sharding over output rows, 8 rows/core, memory-regime):
  - Groups of 4 consecutive x positions share one full-array matmul: the
    contraction is the UNION of the 4 patch windows, planes (c, dy, dx') with
    dx' in 0..7 -> 5*3*8 = 120 partitions.  The stationary [120, 128] holds
    all 4 positions' weights ((j,o) columns, structural zeros where dx'-j is
    outside 0..4), so each [128, 128] matmul output is fully useful:
    out[(j,o), b] for 4 x-positions at once.  15 matmuls per row, 120 per
    core, N=128 streaming.  Bias is added on the host after gathering.
  - dy is handled with a mod-5 ring of row-slots (24 planes each); the per-row
    dy rotation is folded into the host-side W slab layout (np.roll), so the
    device always reads xp[0:120] as one fixed partition range.
  - All input HBM traffic is a few wide DMAs: initial ring fill [120, 3840B]
    on the gpsimd SWDGE queue (the fast path, ~300GB/s) together with the W
    slab in four [128, 7680B] row-pair DMAs; the future-row slabs ride the
    HWDGE queues.  Ring advances are SBUF->SBUF copies from the staged future
    slabs, chunk-gated in two halves per row on the sync queue.
  - Output is bf16 (host upcasts): per-row [128, 3840B] stores, rows 0-4 and
    7 (split in two) on gpsimd, rows 5-6 on the HWDGE queues.  PSUM
    evacuation alternates vector/scalar engines.
  - Dummy matmuls on a scratch tile run during the initial DMA fill to keep
    the PE busy.
"""

import numpy as np

B, C, H, WIDTH = 128, 3, 64, 64
KH = KW = 5
RY = RX = 60
O = 32
NCORES = 8
RPC = 8             # output rows computed per core (8*8=64, last 4 dropped)
INR = RPC + KH - 1  # 12 input rows per core
PADH = NCORES * RPC + KH - 1  # 68
NG = 15             # groups of 4 x-positions per row
NPL = C * 8         # 24 planes per ring slot (c, dx' in 0..7)
KP = KH * NPL       # 120 contraction partitions
FU = NG * B         # 1920 free elems per plane (g, b)
CHUNKS = ((0, 4), (4, 4), (8, 4), (12, 3))  # (first group, n groups) per PSUM chunk
NWARM = 24

_cache = {}


def _build():
    import concourse.bass as bass
    import concourse.bacc as bacc
    import concourse.tile as tile
    import concourse.mybir as mybir

    f32 = mybir.dt.float32
    din = mybir.dt.bfloat16
    nc = bacc.Bacc("TRN2", target_bir_lowering=False, debug=False,
                   num_devices=NCORES)
    ui_d = nc.dram_tensor("ui", [KP, FU], din, kind="ExternalInput")
    ufa_d = nc.dram_tensor("ufa", [KP, FU], din, kind="ExternalInput")
    ufb_d = nc.dram_tensor("ufb", [2 * NPL, FU], din, kind="ExternalInput")
    w_d = nc.dram_tensor("w", [128, RPC * NG * B], din, kind="ExternalInput")
    oc_d = nc.dram_tensor("oc", [RPC, 4, O, NG, B], din, kind="ExternalOutput")

    with tile.TileContext(nc) as tc:
        with (
            tc.tile_pool(name="const", bufs=1) as cpool,
            tc.tile_pool(name="os", bufs=3) as opool,
            tc.tile_pool(name="ps", bufs=4, space=bass.MemorySpace.PSUM) as ppool,
            tc.tile_pool(name="pw", bufs=1, space=bass.MemorySpace.PSUM) as wpool,
        ):
            xp = cpool.tile([KP, FU], din)            # ring planes
            ufa = cpool.tile([KP, FU], din)           # future rows 5-9
            ufb = cpool.tile([2 * NPL, FU], din)      # future rows 10-11
            ws = cpool.tile([128, RPC * NG * B], din)
            dm = cpool.tile([1, 512], din)            # warmup operand

            nc.gpsimd.dma_start(xp[:], ui_d[:])
            for p in range(4):  # W row-pairs on the gpsimd SWDGE queue
                c0, c1 = p * 2 * NG * B, (p + 1) * 2 * NG * B
                nc.gpsimd.dma_start(ws[:, c0:c1], w_d[:, c0:c1])
            nc.scalar.dma_start(ufa[:], ufa_d[:])
            nc.sync.dma_start(ufb[:], ufb_d[:])

            # PE warmup: keep the array busy during the fill so HAM
            # un-throttles the PE clock before the first real matmul
            nc.vector.memset(dm[:], 1.0)
            pw = wpool.tile([1, 512], f32)
            for _ in range(NWARM):
                nc.tensor.matmul(pw[0:1, :], dm[:, 0:1], dm[:, :])

            for k in range(RPC):
                ot = opool.tile([128, FU], din)
                oc_k = oc_d[k].rearrange("j o g b -> (j o) (g b)")
                st_eng = nc.gpsimd if k < 5 else (nc.sync, nc.scalar, nc.sync)[k - 5]
                for ci, (g0, gn) in enumerate(CHUNKS):
                    pt = ppool.tile([128, 4 * B], f32)
                    for gg in range(gn):
                        g = g0 + gg
                        nc.tensor.matmul(
                            pt[:, gg * B:(gg + 1) * B],
                            ws[0:KP, (k * NG + g) * B:(k * NG + g + 1) * B],
                            xp[:, g * B:(g + 1) * B],
                        )
                    if ci % 2 == 0:
                        nc.vector.tensor_copy(
                            ot[:, g0 * B:(g0 + gn) * B], pt[:, :gn * B])
                    else:
                        nc.scalar.copy(
                            ot[:, g0 * B:(g0 + gn) * B], pt[:, :gn * B])
                        f0, f1 = (0, 8 * B) if ci == 1 else (8 * B, FU)
                        if k + KH < INR:
                            # ring advance: slot k%5 <- input row k+5, copied
                            # SBUF->SBUF from the staged future slab
                            s = k % KH
                            src = (ufa[s * NPL:(s + 1) * NPL, f0:f1]
                                   if k + KH < 2 * KH else
                                   ufb[(k - KH) * NPL:(k - KH + 1) * NPL, f0:f1])
                            nc.sync.dma_start(
                                xp[s * NPL:(s + 1) * NPL, f0:f1], src)
                        st_eng.dma_start(oc_k[:, f0:f1], ot[:, f0:f1])

    nc.compile()
    return nc


def _get_nc():
    if "nc" not in _cache:
        _cache["nc"] = _build()
    return _cache["nc"]


def _prep_inputs(x, W, b):
    import ml_dtypes
    bf = ml_dtypes.bfloat16
    x = np.asarray(x, np.float32)
    W = np.asarray(W, np.float32)
    xh = np.zeros((PADH, C, WIDTH, B), np.float32)
    xh[:H] = x.transpose(2, 1, 3, 0)  # [row, c, w, batch]
    # union planes: U[row, (c,dx'), (g,b)] = xh[row, c, 4g+dx', b], dx' in 0..7
    U = np.zeros((PADH, C, 8, NG, B), np.float32)
    for dxp in range(8):
        U[:, :, dxp] = xh[:, :, dxp::4][:, :, :NG]
    U = U.reshape(PADH, NPL, FU).astype(bf)

    W5 = W.reshape(RY, RX, O, C, KH, KW)
    in_maps = []
    for i in range(NCORES):
        nk = min(RPC, RY - RPC * i)
        W5c = np.zeros((RPC, NG, 4, O, C, KH, KW), np.float32)
        W5c[:nk] = W5[RPC * i:RPC * i + nk].reshape(nk, NG, 4, O, C, KH, KW)
        A = W5c.transpose(5, 4, 6, 0, 1, 2, 3)  # [dy, c, dx, k, g, j, o]
        D = np.zeros((KH, C, 8, RPC, NG, 4, O), np.float32)
        for j in range(4):
            D[:, :, j:j + KW, :, :, j, :] = A[:, :, :, :, :, j, :]
        S = np.empty_like(D)  # slot rm holds dy=(rm-k)%5 -> roll dy by k
        for k in range(RPC):
            S[:, :, :, k] = np.roll(D[:, :, :, k], k, axis=0)
        wslab = np.zeros((128, RPC * NG * 4 * O), np.float32)
        wslab[:KP] = S.reshape(KP, -1)

        Uc = U[RPC * i:RPC * i + INR]  # [12, 24, FU]
        in_maps.append({
            "ui": np.ascontiguousarray(Uc[:KH].reshape(KP, FU)),
            "ufa": np.ascontiguousarray(Uc[KH:2 * KH].reshape(KP, FU)),
            "ufb": np.ascontiguousarray(Uc[2 * KH:].reshape(2 * NPL, FU)),
            "w": wslab.astype(bf),
        })
    return in_maps


def kernel(x, W, b):
    from concourse.bass_utils import run_bass_kernel_spmd

    nc = _get_nc()
    in_maps = _prep_inputs(x, W, b)
    br = run_bass_kernel_spmd(nc, in_maps, list(range(NCORES)),
                              **_cache.get("run_kwargs", {}))
    _cache["last_run"] = br
    oc = np.stack([np.asarray(br.results[i]["oc"]) for i in range(NCORES)])
    oc = oc.reshape(NCORES * RPC, 4, O, NG, B).astype(np.float32)
    out = oc.transpose(4, 2, 0, 3, 1).reshape(B, O, NCORES * RPC, RX)
    out = out[:, :, :RY, :] + np.asarray(b, np.float32).transpose(2, 0, 1)[None]
    return np.ascontiguousarray(out)
